# revision 1
# baseline (speedup 1.0000x reference)
"""BERT self-attention forward on 8 Trainium2 NeuronCores.

Host shards batch (4) x head-group (2 x 8 heads) across 8 cores, handing each
core pre-transposed fp16 operands (contraction-dim major) as part of shard
layout; per-core outputs [S, 512] are gathered back into [B, S, D].

Per-core pipeline (S=2048, D=1024, 8 local heads of HD=64):
  - projections on PE (fp16, fp32 accum): Q^T/K^T in [e, s] layout; V in
    natural [s, e] layout interleaved per 32-column quadrant with constant-1
    columns (col 32i is ones, cols 32i+1..31 carry V columns 31i..31i+30).
  - scores computed transposed, S^T[k, q] = K^T.T @ Q^T, two heads per matmul
    pair via PE row tiling (hd=64 contraction each).
  - exp on ScalarE straight from PSUM; additive mask rides the per-partition
    activation bias, the 1/sqrt(64) scale rides the activation scale.
  - ctx^T accumulated over k with lhsT = interleaved [ones|V] (M=96), so each
    32-row quadrant of the accumulator carries the softmax denominator row.
  - tail: fp16 copy, DVE 32x32 block-transpose, reciprocal of the denominator
    plane (present on every quadrant), broadcast-AP multiply, and output DMAs
    whose access patterns undo the 32x32 block permutation.

The target hardware accepts at most ONE sync wait per PE Matmult (the MM ISA
struct has a single wait slot), so dependencies are funneled two ways: the
kernel routes DRAM loads through DVE staging copies / re-stages the mask
through ScalarE / pre-observes ticks with tiny absorber instructions, and the
program is built as bacc.Bacc so finalize() runs the
move_matmul_waits_to_ldweights + generate_event_semaphores passes that
legalize any remaining multi-wait instructions.

attention_mask is applied exactly; q/k/v bias paths compile in only when the
biases are nonzero (they are zero in this problem's input spec).
"""

import sys

sys.path.insert(0, "/opt/trn_rl_repo")

from contextlib import ExitStack

import numpy as np

import concourse.bass as bass
import concourse.bacc as bacc
import concourse.tile as tile
from concourse import mybir
from concourse.bass_utils import run_bass_kernel_spmd

F32 = mybir.dt.float32
F16 = mybir.dt.float16
F8 = mybir.dt.float8e4  # TRN e4m3, max +-240; all operands here stay < ~10
DR = mybir.MatmulPerfMode.DoubleRow

PART = 128
S = 2048
D = 1024
E = 512  # per-core output features (8 heads x 64)
HD = 64
NHL = 8  # local heads per core
NEI = E // PART  # 4 e-tiles
NDI = D // PART  # 8 d-tiles
NSB = S // 512  # 4 s-blocks for projections
NKT = S // PART  # 16 k-tiles
NQB = S // 512  # 4 q-blocks
VW = 96  # V columns per head: 3 quadrants of [ones | 31 V columns]

B = 4
N_CORES = 8


def build_program(with_qkbias: bool = False, with_vbias: bool = False):
    # Bacc (not bass.Bass): its finalize() runs move_matmul_waits_to_ldweights
    # + generate_event_semaphores, which legalize instructions that would
    # otherwise carry two sync waits (PE Matmult accepts only one).
    nc = bacc.Bacc()

    xT_d = nc.dram_tensor("xT", [D, S], F16, kind="ExternalInput")
    wqT_d = nc.dram_tensor("wqT", [D, E], F16, kind="ExternalInput")
    wkT_d = nc.dram_tensor("wkT", [D, E], F16, kind="ExternalInput")
    wvT_d = nc.dram_tensor("wvT", [D, E], F16, kind="ExternalInput")
    bq_d = nc.dram_tensor("bq", [E], F32, kind="ExternalInput")
    bk_d = nc.dram_tensor("bk", [E], F32, kind="ExternalInput")
    bv_d = nc.dram_tensor("bv", [E], F32, kind="ExternalInput")
    mask_d = nc.dram_tensor("mask", [S], F32, kind="ExternalInput")
    out_d = nc.dram_tensor("out", [S, E], F32, kind="ExternalOutput")
    oscr_d = nc.dram_tensor("oscr", [4 * NQB * 2, 3, 32, 512], F32)

    with tile.TileContext(nc) as tc, ExitStack() as ctx:
        persist = ctx.enter_context(tc.tile_pool(name="persist", bufs=1))
        ldpool = ctx.enter_context(tc.tile_pool(name="ld", bufs=7))
        qk_ps = ctx.enter_context(tc.tile_pool(name="qkps", bufs=2, space="PSUM"))
        stg_ps = ctx.enter_context(tc.tile_pool(name="stgps", bufs=2, space="PSUM"))
        c_ps = ctx.enter_context(tc.tile_pool(name="cps", bufs=2, space="PSUM"))
        ppool = ctx.enter_context(tc.tile_pool(name="pp", bufs=3))
        tailp = ctx.enter_context(tc.tile_pool(name="tail", bufs=2))

        xT = persist.tile([PART, NDI, S], F16)  # X^T: [d%128, d//128, s]
        wqT = persist.tile([PART, NDI, E], F16)  # W^T: [d%128, d//128, e]
        wkT = persist.tile([PART, NDI, E], F16)
        wvT = persist.tile([PART, NDI, E], F16)
        qT = persist.tile([PART, NEI, S], F16)  # Q^T: [e%128, e//128, s]
        kT = persist.tile([PART, NEI, S], F16)
        vSB = persist.tile([PART, NKT, NHL * VW], F16)  # interleaved [ones|V]
        mask_raw = persist.tile([PART, NKT], F32)
        mask_sb = persist.tile([PART, NKT], F32)
        scr = persist.tile([1, 16], F16)  # absorber scratch

        nc.sync.dma_start(
            out=mask_raw, in_=mask_d[:].rearrange("(k p) -> p k", p=PART)
        )
        # re-stage the mask through ScalarE so the exps' mask dependency is
        # an ACT-local tick
        nc.scalar.copy(out=mask_sb, in_=mask_raw)

        if with_qkbias:
            bq_sb = persist.tile([PART, NEI], F32)
            bk_sb = persist.tile([PART, NEI], F32)
            nc.sync.dma_start(
                out=bq_sb, in_=bq_d[:].rearrange("(e p) -> p e", p=PART)
            )
            nc.sync.dma_start(
                out=bk_sb, in_=bk_d[:].rearrange("(e p) -> p e", p=PART)
            )
        else:
            bq_sb = bk_sb = None
        if with_vbias:
            # bv in the tail's block-transposed layout, per quadrant triple:
            # bvb[32a+c, hl, j] = bv[64*hl + 31a + (j-1)] (j>=1), 0 for j=0
            bvb = persist.tile([PART, NHL, 32], F32)
            nc.vector.memset(bvb, 0.0)
            for a in range(3):
                w = 31 if a < 2 else 2
                nc.gpsimd.dma_start(
                    out=bvb[32 * a : 32 * a + 32, :, 1 : 1 + w],
                    in_=bass.AP(
                        tensor=bv_d,
                        offset=31 * a,
                        ap=[[0, 32], [HD, NHL], [1, w]],
                    ),
                )

            def bv_bcast(hl, a):
                base = bvb[32 * a : 32 * a + 32, hl, :]
                return bass.AP(
                    tensor=base.tensor,
                    offset=base.offset,
                    ap=[list(base.ap[0]), [0, 16], list(base.ap[1])],
                )

        # vSB: zero everything (junk V slots stay 0), then the ones columns
        nc.vector.memset(vSB, 0.0)
        nc.vector.memset(
            vSB.rearrange("p kt (m j) -> p kt m j", j=32)[:, :, :, 0:1], 1.0
        )

        # --- loads: DRAM -> staging -> DVE copy, so consumers' data deps are
        # DVE-local ---
        for w_d, wT in ((wkT_d, wkT), (wqT_d, wqT), (wvT_d, wvT)):
            wst = ldpool.tile([PART, NDI * E], F16, tag="ldst", name="wst")
            nc.sync.dma_start(
                out=wst.rearrange("p (di e) -> p di e", di=NDI),
                in_=w_d[:].rearrange("(di p) e -> p di e", p=PART),
            )
            nc.vector.tensor_copy(
                out=wT, in_=wst.rearrange("p (di e) -> p di e", di=NDI)
            )

        def load_x_block(sb):
            xst = ldpool.tile([PART, NDI * E], F16, tag="ldst", name="xst")
            nc.sync.dma_start(
                out=xst.rearrange("p (di s) -> p di s", di=NDI),
                in_=xT_d[:, sb * 512 : (sb + 1) * 512].rearrange(
                    "(di p) s -> p di s", p=PART
                ),
            )
            nc.vector.tensor_copy(
                out=xT[:, :, sb * 512 : (sb + 1) * 512],
                in_=xst.rearrange("p (di s) -> p di s", di=NDI),
            )

        def proj_block(wT, ei, sb0, nsb, dstT, bias_sb):
            psums = [
                qk_ps.tile([PART, 512], F32, tag="qkpsum", name="qkpsum")
                for _ in range(nsb)
            ]
            for di in range(NDI):
                for j in range(nsb):
                    sb = sb0 + j
                    nc.tensor.matmul(
                        psums[j],
                        lhsT=wT[:, di, ei * 128 : (ei + 1) * 128],
                        rhs=xT[:, di, sb * 512 : (sb + 1) * 512],
                        start=(di == 0),
                        stop=(di == NDI - 1),
                    )
            for j in range(nsb):
                sb = sb0 + j
                dst = dstT[:, ei, sb * 512 : (sb + 1) * 512]
                if bias_sb is None:
                    nc.vector.tensor_copy(out=dst, in_=psums[j])
                else:
                    nc.vector.tensor_scalar_add(
                        out=dst, in0=psums[j], scalar1=bias_sb[:, ei : ei + 1]
                    )

        for sbp in range(2):
            for j in range(2):
                load_x_block(sbp * 2 + j)
            proj_block(wkT, 0, sbp * 2, 2, kT, bk_sb)
            proj_block(wqT, 0, sbp * 2, 2, qT, bq_sb)

        # V projection directly into the interleaved [ones|V] layout
        for st in range(NKT):
            vps = qk_ps.tile([PART, 512], F32, tag="qkpsum", name="vps")
            for di in range(NDI):
                nc.tensor.matmul(
                    vps,
                    lhsT=xT[:, di, st * 128 : (st + 1) * 128],
                    rhs=wvT[:, di, :],
                    start=(di == 0),
                    stop=(di == NDI - 1),
                )
            vdst = vSB[:, st, :].rearrange("p (hl m j) -> p hl m j", m=3, j=32)
            vsrc = vps.rearrange("p (hl v) -> p hl v", v=HD)
            # quadrants 0/1: V cols 31a..31a+30 into slots j=1..31
            nc.vector.tensor_copy(
                out=vdst[:, :, 0:2, 1:32],
                in_=bass.AP(
                    tensor=vsrc.tensor,
                    offset=vsrc.offset,
                    ap=[list(vsrc.ap[0]), list(vsrc.ap[1]), [31, 2], [1, 31]],
                ),
            )
            # quadrant 2: V cols 62..63 into slots j=1..2
            nc.vector.tensor_copy(
                out=vdst[:, :, 2:3, 1:3],
                in_=bass.AP(
                    tensor=vsrc.tensor,
                    offset=vsrc.offset + 62,
                    ap=[list(vsrc.ap[0]), list(vsrc.ap[1]), [31, 1], [1, 2]],
                ),
            )

        def attn(hp, first):
            for qb in range(NQB):
                cps = [
                    c_ps.tile([VW, 512], F32, tag="cps", name="cps")
                    for _ in range(2)
                ]
                for kt in range(NKT):
                    sps = stg_ps.tile([PART, 1024], F32, tag="sps")
                    if kt == 0 and qb == 0:
                        # absorbers: pre-observe the fresh qT/kT DVE ticks on
                        # PE without ever carrying two cross-engine waits
                        nc.vector.tensor_copy(
                            out=scr[:, 0:4], in_=qT[0:1, hp, 0:2048:512]
                        )
                        nc.vector.tensor_copy(
                            out=scr[:, 4:8], in_=kT[0:1, hp, 0:2048:512]
                        )
                        nc.tensor.matmul(
                            sps[0:1, 0:1],
                            lhsT=xT[0:1, 0, 0:1],
                            rhs=xT[0:1, 0, 0:1],
                            start=True,
                            stop=True,
                        )
                        nc.tensor.matmul(
                            sps[0:1, 1:2],
                            lhsT=scr[0:1, 0:1],
                            rhs=scr[0:1, 0:1],
                            start=True,
                            stop=True,
                        )
                    for h in range(2):
                        pr = 64 * h
                        nc.tensor.matmul(
                            sps[:, h * 512 : (h + 1) * 512],
                            lhsT=kT[pr : pr + 64, hp, kt * 128 : (kt + 1) * 128],
                            rhs=qT[pr : pr + 64, hp, qb * 512 : (qb + 1) * 512],
                            start=True,
                            stop=True,
                        )
                    pb = ppool.tile([PART, 1024], F16, tag="pb")
                    nc.scalar.activation(
                        out=pb,
                        in_=sps,
                        func=mybir.ActivationFunctionType.Exp,
                        bias=mask_sb[:, kt : kt + 1],
                        scale=0.125,
                    )
                    for h in range(2):
                        hl = 2 * hp + h
                        if kt == 0:
                            # absorb the C-slot WAR (DVE) ahead of the real
                            # start=True matmul; its garbage is cleared by it
                            nc.tensor.matmul(
                                cps[h][0:1, 0:1],
                                lhsT=xT[0:1, 0, 0:1],
                                rhs=xT[0:1, 0, 0:1],
                                start=True,
                                stop=True,
                            )
                        nc.tensor.matmul(
                            cps[h],
                            lhsT=vSB[:, kt, hl * VW : (hl + 1) * VW],
                            rhs=pb[:, h * 512 : (h + 1) * 512],
                            start=(kt == 0),
                            stop=(kt == NKT - 1),
                        )
                for h in range(2):
                    hl = 2 * hp + h
                    cb = tailp.tile([VW, 512], F16, tag="cb")
                    nc.vector.tensor_copy(out=cb, in_=cps[h])
                    ct = tailp.tile([VW, 512], F16, tag="ct")
                    nc.vector.transpose(out=ct, in_=cb)
                    # ct[32a+c, 32b+r] = C[32a+r, 32b+c]; the r=0 plane of
                    # every quadrant is rowsum[32b+c]
                    rqt = tailp.tile([VW, 16, 1], F32, tag="rqt")
                    nc.vector.reciprocal(
                        out=rqt,
                        in_=ct.rearrange("p (b r) -> p b r", r=32)[:, :, 0:1],
                    )
                    ob = tailp.tile([VW, 512], F32, tag="ob")
                    for a in range(3):
                        sl = slice(32 * a, 32 * a + 32)
                        rq_base = rqt[sl, :, 0]
                        rq_bcast = bass.AP(
                            tensor=rq_base.tensor,
                            offset=rq_base.offset,
                            ap=[
                                list(rq_base.ap[0]),
                                list(rq_base.ap[1]),
                                [0, 32],
                            ],
                        )
                        nc.vector.tensor_mul(
                            out=ob[sl, :].rearrange("p (b r) -> p b r", r=32),
                            in0=ct[sl, :].rearrange("p (b r) -> p b r", r=32),
                            in1=rq_bcast,
                        )
                        if with_vbias:
                            nc.vector.tensor_add(
                                out=ob[sl, :].rearrange(
                                    "p (b r) -> p b r", r=32
                                ),
                                in0=ob[sl, :].rearrange("p (b r) -> p b r", r=32),
                                in1=bv_bcast(hl, a),
                            )
                    # single SBUF->DRAM dump keeps this ob slot's accessor
                    # set to {3 DVE mults, 1 DMA}, so the recycled slot's
                    # release carries one queue sem; the block-unscramble
                    # happens DRAM->DRAM (DMA copies take multiple waits)
                    par = (hp * NQB + qb) * 2 + h
                    for a in range(3):
                        w = 31 if a < 2 else 2
                        nc.sync.dma_start(
                            out=oscr_d[par, a], in_=ob[32 * a : 32 * a + 32, :]
                        )
                        nc.sync.dma_start(
                            out=out_d[
                                qb * 512 : (qb + 1) * 512,
                                hl * HD + 31 * a : hl * HD + 31 * a + w,
                            ].rearrange("(b c) j -> c b j", c=32),
                            in_=oscr_d[par, a].rearrange(
                                "c (b r) -> c b r", r=32
                            )[:, :, 1 : 1 + w],
                        )

        attn(0, True)
        for hp in range(1, 4):
            for sbp in range(2):
                proj_block(wkT, hp, sbp * 2, 2, kT, bk_sb)
                proj_block(wqT, hp, sbp * 2, 2, qT, bq_sb)
            attn(hp, False)

    nc.finalize()
    return nc


_NC_CACHE = {}


def _get_nc(with_qkbias: bool, with_vbias: bool):
    key = (with_qkbias, with_vbias)
    if key not in _NC_CACHE:
        _NC_CACHE[key] = build_program(*key)
    return _NC_CACHE[key]


def _make_in_maps(hidden_states, attention_mask, Wq, bq, Wk, bk, Wv, bv):
    wqT = {}
    wkT = {}
    wvT = {}
    for g in range(2):
        sl = slice(g * E, (g + 1) * E)
        wqT[g] = np.ascontiguousarray(Wq[sl].T.astype(np.float16))
        wkT[g] = np.ascontiguousarray(Wk[sl].T.astype(np.float16))
        wvT[g] = np.ascontiguousarray(Wv[sl].T.astype(np.float16))
    xT = {}
    for b in range(B):
        xT[b] = np.ascontiguousarray(hidden_states[b].T.astype(np.float16))

    in_maps = []
    for c in range(N_CORES):
        b, g = c // 2, c % 2
        sl = slice(g * E, (g + 1) * E)
        in_maps.append(
            {
                "xT": xT[b],
                "wqT": wqT[g],
                "wkT": wkT[g],
                "wvT": wvT[g],
                "bq": np.ascontiguousarray(bq[sl]),
                "bk": np.ascontiguousarray(bk[sl]),
                "bv": np.ascontiguousarray(bv[sl]),
                "mask": np.ascontiguousarray(attention_mask[b, 0, 0, :]),
            }
        )
    return in_maps


def _prep(inputs):
    return {k: np.asarray(v, dtype=np.float32) for k, v in inputs.items()}


def _run(ins, trace):
    nc = _get_nc(
        bool(np.any(ins["bq"])) or bool(np.any(ins["bk"])),
        bool(np.any(ins["bv"])),
    )
    in_maps = _make_in_maps(
        ins["hidden_states"], ins["attention_mask"], ins["Wq"], ins["bq"],
        ins["Wk"], ins["bk"], ins["Wv"], ins["bv"],
    )
    return run_bass_kernel_spmd(
        nc, in_maps, core_ids=list(range(N_CORES)), trace=trace
    )


def run_traced(inputs):
    """Run once with NTFF tracing; returns BassKernelResults (test.py helper)."""
    return _run(_prep(inputs), True)


def _jax_fallback(ins):
    """Plain-jax attention on the 8 NeuronCores (one batch x head-group shard
    per device); correctness fallback if the Bass path fails to compile."""
    import jax
    import jax.numpy as jnp

    devs = jax.devices()[:N_CORES]

    @jax.jit
    def shard_attn(x, wqt, wkt, wvt, bq, bk, bv, mask):
        # fp16 matmul operands (full-rate on the PE array), fp32 accumulation
        f32 = jnp.float32
        q = (
            jnp.matmul(x, wqt, preferred_element_type=f32) + bq
        ).reshape(S, NHL, HD).transpose(1, 0, 2)
        k = (
            jnp.matmul(x, wkt, preferred_element_type=f32) + bk
        ).reshape(S, NHL, HD).transpose(1, 0, 2)
        v = (
            jnp.matmul(x, wvt, preferred_element_type=f32) + bv
        ).reshape(S, NHL, HD).transpose(1, 0, 2)
        s = jnp.einsum(
            "hqd,hkd->hqk",
            q.astype(jnp.float16),
            k.astype(jnp.float16),
            preferred_element_type=f32,
        ) / np.sqrt(np.float32(HD))
        p = jax.nn.softmax(s + mask[None, None, :], axis=-1)
        c = jnp.einsum(
            "hqk,hkd->hqd",
            p.astype(jnp.float16),
            v.astype(jnp.float16),
            preferred_element_type=f32,
        )
        # fp16 on the wire; host casts back to f32 (halves the fetch)
        return c.transpose(1, 0, 2).reshape(S, E).astype(jnp.float16)

    # fp16 host-side casts (halves transfer bytes; device would round the
    # same way), weights pre-transposed so the device matmul is x @ W^T
    xh = {b: ins["hidden_states"][b].astype(np.float16) for b in range(B)}
    wh = {}
    for g in range(2):
        sl = slice(g * E, (g + 1) * E)
        wh[g] = [
            np.ascontiguousarray(w[sl].T.astype(np.float16))
            for w in (ins["Wq"], ins["Wk"], ins["Wv"])
        ]
    from concurrent.futures import ThreadPoolExecutor

    def _one(c):
        b, g = c // 2, c % 2
        sl = slice(g * E, (g + 1) * E)
        args = [
            xh[b], *wh[g], ins["bq"][sl], ins["bk"][sl], ins["bv"][sl],
            ins["attention_mask"][b, 0, 0, :],
        ]
        args = [jax.device_put(a, devs[c]) for a in args]
        return shard_attn(*args)

    # overlap per-array transfer round-trips across the 8 cores
    with ThreadPoolExecutor(max_workers=N_CORES) as ex:
        outs = list(ex.map(_one, range(N_CORES)))
    out = np.empty((B, S, D), np.float32)
    for c in range(N_CORES):
        b, g = c // 2, c % 2
        out[b, :, g * E : (g + 1) * E] = np.asarray(outs[c]).astype(np.float32)
    return out


# The Bass/Tile path is the default. (The walrus single-sync-wait limit on
# PE Matmult is handled by building with bacc.Bacc + finalize(), whose
# move_matmul_waits_to_ldweights / generate_event_semaphores passes legalize
# multi-wait instructions.) Set BASS_ATTN=0 to force the jax fallback.
import os

_BASS_BROKEN = os.environ.get("BASS_ATTN", "1") != "1"


def kernel(hidden_states, attention_mask, Wq, bq, Wk, bk, Wv, bv):
    global _BASS_BROKEN
    ins = _prep(
        {
            "hidden_states": hidden_states,
            "attention_mask": attention_mask,
            "Wq": Wq, "bq": bq, "Wk": Wk, "bk": bk, "Wv": Wv, "bv": bv,
        }
    )
    if not _BASS_BROKEN:
        try:
            res = _run(ins, False)
            out = np.empty((B, S, D), np.float32)
            for c in range(N_CORES):
                b, g = c // 2, c % 2
                out[b, :, g * E : (g + 1) * E] = res.results[c]["out"]
            return out
        except Exception as e:  # compile/runtime failure -> jax fallback
            sys.stderr.write(f"bass path failed ({type(e).__name__}: {e});"
                             " falling back to jax\n")
            _BASS_BROKEN = True
    return _jax_fallback(ins)

